# revision 28
# baseline (speedup 1.0000x reference)
"""AudioWaveAugment Trainium2 kernel.

Reference computation (per sample i of B=128, C=1, T=320000):
  1. g = gains if do_gain<0.7 else 1 ;  x1 = x*g
  2. std = clip(std(x1, ddof=1), 1e-4) ; x2 = x1 + noise*(nmask*std*noise_scales)
  3. low = moving_avg(x2, k=2h+1, zero pad) ; out = {x2 | low | x2-low} per
     (do_filter, low_coin) coins.

Strategy: pure data parallel over 8 NeuronCores, 16 samples per core.

Wire format: fp16 both directions (tolerance is 2e-2 relative to max|out|,
so fp16 roundoff ~1e-3 is far inside budget).  Layout is partition-
interleaved: tile[p, j] = x[j*128 + p], prepared host-side, so that a
windowed (moving-average) sum along time becomes a *banded matrix multiply
over the partition axis* executed on the otherwise-idle PE engine:

  out[p, j] = sum_q W0[q,p] x2[q, j] + WL[q,p] x2[q, j-1] + WU[q,p] x2[q, j+1]

with W0 = s*band(|q-p|<=h) + m*I, WL/WU the corner matrices for windows
crossing a column boundary, s = +-1/k, m in {0,1}.  The per-sample filter
coefficients (including the "m*x2 +" term of the high-pass) are entirely
encoded in the host-built matrices, so low-pass / high-pass / no-filter all
run the same instruction sequence, and PSUM accumulates the final output.

Samples are classified by (noise on, filter on) into 4 classes; each core
gets an identical slot pattern (SPMD), with lighter samples padding heavier
slots when counts don't divide evenly.  Noise DMA is skipped for slots that
don't need it.  std is computed on device from a 1/4-strided subsample
(Square+accum on ACT, ones-matmul partition broadcast on PE); the mean^2
term of the variance is negligible (|mean| ~ 2e-3) and dropped.

Engine budget per core (target):  DVE ~30us (fp16 tensor_scalar 4x /
tensor_tensor 2x), ACT ~25us (Square+accum, sqrt, PSUM->SBUF copies),
PE ~25-50us (banded matmuls), DMA in 15.4MB / out 10.2MB.
"""

import ml_dtypes
import numpy as np
from contextlib import ExitStack

import concourse.bass as bass
import concourse.bacc as bacc
import concourse.tile as tile
import concourse.mybir as mybir
from concourse.bass_utils import run_bass_kernel_spmd

N_CORES = 8
B, T = 128, 320000
P = 128
NS = B // N_CORES          # samples (slots) per core = 16
F = T // P                 # free size per partition = 2500
FE = F + 4                 # x2e: 2 left pad cols + F + 2 right pad cols
F16 = mybir.dt.float16
BF16 = mybir.dt.bfloat16
F32 = mybir.dt.float32
CHUNK = 512                # psum bank = 512 fp32
FH = F // 2                # 1250: D computed/copied in two psum halves
NCHH = (FH + CHUNK - 1) // CHUNK  # 3 chunks per half: 512+512+226

GAIN_PROB, NOISE_PROB, FILTER_PROB = 0.7, 0.5, 0.35
NSUB = 4                   # std computed from x[::NSUB] subsample
FSUB = F // NSUB           # 625 elems/partition

# exec info of the last run (for test harnesses); not used by grading
LAST_RUN = {}


def weave(counts):
    """Slot order: band slots (bf/fo alternating) first so PE starts early,
    then noise-only, then plain slots draining the tail."""
    order = []
    bf, fo = counts.get("bf", 0), counts.get("fo", 0)
    for i in range(bf + fo):
        if (i % 2 == 0 and fo > 0) or bf == 0:
            order.append("fo"); fo -= 1
        else:
            order.append("bf"); bf -= 1
    order += ["no"] * counts.get("no", 0)
    order += ["nn"] * counts.get("nn", 0)
    return order


def build_program(cfg):
    """cfg: tuple of NS class strings in slot order ('bf','no','fo','nn')."""
    noise_slots = [i for i, c in enumerate(cfg) if c in ("bf", "no")]
    band_slots = [i for i, c in enumerate(cfg) if c in ("bf", "fo")]
    n_noise = len(noise_slots)
    n_band = len(band_slots)
    nidx = {s: j for j, s in enumerate(noise_slots)}
    bidx = {s: j for j, s in enumerate(band_slots)}

    nelem = float(T)
    nsub = float(FSUB * P)
    # ct = sqrt(Qb * nm2cq_ap); host folds nm^2 * N/(N'*(N-1)) into the scalar
    nc = bacc.Bacc("TRN2", debug=False, enable_asserts=False,
                   num_devices=N_CORES)

    x_d = nc.dram_tensor("x_sh", [NS, P, F], BF16, kind="ExternalInput").ap()
    n_d = None
    xs_d = None
    if n_noise:
        n_d = nc.dram_tensor("n_sh", [max(n_noise, 1), P, F], BF16,
                             kind="ExternalInput").ap()
        xs_d = nc.dram_tensor("xsub", [P, n_noise * FSUB], BF16,
                              kind="ExternalInput").ap()
    w_d = None
    if n_band:
        w_d = nc.dram_tensor("w_sh", [P, n_band * 3 * P], BF16,
                             kind="ExternalInput").ap()
    s_d = nc.dram_tensor("scal", [P, NS + max(n_noise, 1)], F32,
                         kind="ExternalInput").ap()
    y_d = nc.dram_tensor("y_sh", [NS, P, F], BF16,
                         kind="ExternalOutput").ap()

    Act = mybir.ActivationFunctionType
    Op = mybir.AluOpType

    with tile.TileContext(nc) as tc, ExitStack() as ctx:
        cpool = ctx.enter_context(tc.tile_pool(name="const", bufs=1))
        ssb = cpool.tile([P, NS + max(n_noise, 1)], F32, name="ssb")
        nc.sync.dma_start(ssb[:], s_d)
        wsb = None
        if n_band:
            wsb = cpool.tile([P, n_band * 3 * P], BF16, name="wsb")
            nc.sync.dma_start(wsb[:], w_d)
        ones = cpool.tile([P, P], F32, name="ones")
        nc.gpsimd.memset(ones[:], 1.0)

        xpool = ctx.enter_context(tc.tile_pool(name="xp", bufs=2))
        npool = ctx.enter_context(tc.tile_pool(name="np", bufs=2))
        wpool = ctx.enter_context(tc.tile_pool(name="wp", bufs=2))
        opool = ctx.enter_context(tc.tile_pool(name="op", bufs=2))
        qpool = ctx.enter_context(tc.tile_pool(name="qp", bufs=2))
        ppool = ctx.enter_context(tc.tile_pool(name="ps", bufs=1,
                                               space="PSUM"))

        st = {}

        def g_ap(k):
            return ssb[:, k:k + 1]

        # ---- stats prologue: ct[:, j] = nm_j * std(g_j * x_j) for all
        # noise slots, from the packed 1/4 subsample.  Decoupled from the
        # main stream so no mid-pipeline ACT<->PE<->DVE stats round trips.
        # Squares run on DVE during the initial DMA fill; the partition
        # broadcast + sqrts are emitted at loop step 2 so the first band
        # slots own the head of the PE queue.
        ct_all = None
        qacc = None
        if n_noise:
            xsub = cpool.tile([P, n_noise * FSUB], BF16, name="xsub")
            nc.sync.dma_start(xsub[:], xs_d)
            qacc = cpool.tile([P, n_noise], F32, name="qacc")
            ct_all = cpool.tile([P, n_noise], F32, name="ct_all")
            for j, k in enumerate(noise_slots):
                sqs = qpool.tile([P, FSUB], BF16, name="sqs", bufs=2)
                xs_j = xsub[:, j * FSUB:(j + 1) * FSUB]
                # Q_j = sum((g*x)^2) over the subsample, on ACT
                nc.scalar.activation(sqs[:], xs_j, Act.Square,
                                     scale=g_ap(k),
                                     accum_out=qacc[:, j:j + 1])

        def stats_back():
            if not n_noise:
                return
            qb = ppool.tile([P, n_noise], F32, name="qb", bufs=1)
            nc.tensor.matmul(qb[:], ones[:], qacc[:], start=True, stop=True)
            for j in range(n_noise):
                nc.scalar.activation(ct_all[:, j:j + 1], qb[:, j:j + 1],
                                     Act.Sqrt,
                                     scale=ssb[:, NS + j:NS + j + 1])

        def ct_ap(k):
            return ct_all[:, nidx[k]:nidx[k] + 1]

        def phA(k):
            """Loads.  g is folded into the band matrices host-side, so 'fo'
            slots DMA straight into the padded matmul operand — zero
            engine ops."""
            cls = cfg[k]
            d = {}
            if cls in ("bf", "fo"):
                xe = wpool.tile([P, FE], BF16, name="xe", bufs=3)
                nc.gpsimd.memset(xe[:, 0:2], 0.0)
                nc.gpsimd.memset(xe[:, F + 2:FE], 0.0)
                d["xe"] = xe
            if cls == "fo":
                nc.sync.dma_start(d["xe"][:, 2:F + 2], x_d[k])
            else:
                xt = xpool.tile([P, F], BF16, name="xt", bufs=4)
                nc.sync.dma_start(xt[:], x_d[k])
                d["xt"] = xt
            if cls in ("bf", "no"):
                nt = npool.tile([P, F], BF16, name="nt", bufs=5)
                nc.sync.dma_start(nt[:], n_d[nidx[k]])
                d["nt"] = nt
            st[k] = d

        def phB(k):
            """out = g*x for plain slots; gx for 'no' slots."""
            cls = cfg[k]
            d = st[k]
            if cls == "nn":
                ot = opool.tile([P, F], BF16, name="ot", bufs=6)
                nc.vector.tensor_scalar_mul(ot[:], d["xt"][:], g_ap(k))
                d["ot"] = ot
            elif cls == "no":
                gx = npool.tile([P, F], BF16, name="gx", bufs=3)
                nc.vector.tensor_scalar_mul(gx[:], d["xt"][:], g_ap(k))
                d["gx"] = gx

        def phC(k):
            """Noise add.  'bf': x2' = x + (ct/g)*noise (g folded into W);
            'no': out = g*x + ct*noise."""
            cls = cfg[k]
            if cls not in ("bf", "no"):
                return
            d = st[k]
            nsc = npool.tile([P, F], BF16, name="nsc", bufs=2)
            nc.vector.tensor_scalar_mul(nsc[:], d["nt"][:], ct_ap(k))
            if cls == "bf":
                nc.vector.tensor_tensor(d["xe"][:, 2:F + 2], nsc[:],
                                        d["xt"][:], Op.add)
            else:  # no
                ot = opool.tile([P, F], BF16, name="ot", bufs=6)
                nc.vector.tensor_tensor(ot[:], nsc[:], d["gx"][:], Op.add)
                d["ot"] = ot

        def phD(k):
            """Banded matmuls: D_psum = (WL|W0|WU) . x2e  (band slots).
            Two psum halves double-buffer against the ACT copies."""
            cls = cfg[k]
            if cls not in ("bf", "fo"):
                return
            d = st[k]
            xe = d["xe"]
            j = bidx[k]
            d["dps"] = []
            for half in range(2):
                h0 = half * FH
                dps = ppool.tile([P, FH], F32, name="dps", bufs=2)
                for c in range(NCHH):
                    c0 = c * CHUNK
                    c1 = min(FH, c0 + CHUNK)
                    for b in range(3):  # 0=WL (col j-1), 1=W0 (j), 2=WU (j+1)
                        w = wsb[:, (3 * j + b) * P:(3 * j + b + 1) * P]
                        nc.tensor.matmul(dps[:, c0:c1], w,
                                         xe[:, h0 + c0 + b + 1:
                                             h0 + c1 + b + 1],
                                         start=(b == 0), stop=(b == 2))
                d["dps"].append(dps)

        def phE(k):
            """PSUM -> out copies (band slots); store."""
            cls = cfg[k]
            d = st.pop(k)
            if cls in ("bf", "fo"):
                ot = opool.tile([P, F], BF16, name="ot", bufs=6)
                for half in range(2):
                    h0 = half * FH
                    nc.scalar.activation(ot[:, h0:h0 + FH],
                                         d["dps"][half][:], Act.Copy)
                d["ot"] = ot
            nc.gpsimd.dma_start(y_d[k], d["ot"][:])

        LB, LC, LD, LE = 1, 2, 3, 5
        for k in range(NS + LE):
            if k < NS:
                phA(k)
            if k == 2:
                stats_back()
            if LB <= k < NS + LB:
                phB(k - LB)
            if LC <= k < NS + LC:
                phC(k - LC)
            if LD <= k < NS + LD:
                phD(k - LD)
            if LE <= k < NS + LE:
                phE(k - LE)

    nc.compile()
    return nc


_PROGRAM_CACHE = {}


def _get_program(cfg):
    if cfg not in _PROGRAM_CACHE:
        _PROGRAM_CACHE[cfg] = build_program(cfg)
    return _PROGRAM_CACHE[cfg]


def _ceil_div(a, b):
    return -(-a // b)


def assign_slots(cls_of):
    """Global sample->slot assignment, identical slot pattern per core.

    Returns (cfg, per_core) where cfg is the slot class tuple and
    per_core[c] is the list of NS sample indices for core c in slot order.
    Samples may be upgraded to heavier slot classes when counts don't
    divide evenly by N_CORES (correct because heavier slots subsume
    lighter behavior via s=0/m=1 band matrices and nm=0).
    """
    pools = {c: [i for i in range(B) if cls_of[i] == c]
             for c in ("bf", "no", "fo", "nn")}

    def take_pads(n, order):
        pads = []
        for c in order:
            while n > 0 and pools[c]:
                pads.append(pools[c].pop())
                n -= 1
        if n > 0:
            raise ValueError("pad pool dry")
        return pads

    A = _ceil_div(len(pools["bf"]), N_CORES) if pools["bf"] else 0
    bf_all = pools["bf"] + take_pads(A * N_CORES - len(pools["bf"]),
                                     ("no", "fo", "nn"))
    pools["bf"] = []
    C = _ceil_div(len(pools["fo"]), N_CORES) if pools["fo"] else 0
    fo_all = pools["fo"] + take_pads(C * N_CORES - len(pools["fo"]), ("nn",))
    pools["fo"] = []
    Bn = _ceil_div(len(pools["no"]), N_CORES) if pools["no"] else 0
    no_all = pools["no"] + take_pads(Bn * N_CORES - len(pools["no"]), ("nn",))
    pools["no"] = []
    D = NS - A - Bn - C
    nn_all = pools["nn"]
    if D < 0 or len(nn_all) != D * N_CORES:
        raise ValueError("slot arithmetic failed")

    counts = {c: n for c, n in
              (("bf", A), ("no", Bn), ("fo", C), ("nn", D)) if n}
    cfg = tuple(weave(counts))
    per_core = []
    for c in range(N_CORES):
        by_cls = {"bf": list(bf_all[c::N_CORES]),
                  "no": list(no_all[c::N_CORES]),
                  "fo": list(fo_all[c::N_CORES]),
                  "nn": list(nn_all[c::N_CORES])}
        per_core.append([by_cls[cl].pop(0) for cl in cfg])
    return cfg, per_core


def kernel(x, gains, noise_scales, noise, do_gain, do_noise, do_filter,
           low_coin, halves, _trace=False):
    x = np.asarray(x, dtype=np.float32)
    noise = np.asarray(noise, dtype=np.float32)
    gains = np.asarray(gains, dtype=np.float32)
    noise_scales = np.asarray(noise_scales, dtype=np.float32)
    do_gain = np.asarray(do_gain, dtype=np.float32)
    do_noise = np.asarray(do_noise, dtype=np.float32)
    do_filter = np.asarray(do_filter, dtype=np.float32)
    low_coin = np.asarray(low_coin, dtype=np.float32)
    halves = np.asarray(halves).astype(np.int64)

    g = np.where(do_gain < GAIN_PROB, gains, np.float32(1.0)).astype(
        np.float32)
    nm = np.where(do_noise < NOISE_PROB, noise_scales,
                  np.float32(0.0)).astype(np.float32)
    filt_on = np.asarray(do_filter < FILTER_PROB)
    lowp = np.asarray(low_coin < 0.5)
    kk = 2 * halves + 1
    s_coef = np.where(filt_on, np.where(lowp, 1.0 / kk, -1.0 / kk),
                      0.0).astype(np.float32)
    m_coef = np.where(filt_on & lowp, 0.0, 1.0).astype(np.float32)
    h_eff = np.where(filt_on, halves, 0).astype(np.int64)

    noise_on = nm > 0
    cls_of = []
    for i in range(B):
        cls_of.append("bf" if (noise_on[i] and filt_on[i]) else
                      "no" if noise_on[i] else
                      "fo" if filt_on[i] else "nn")
    try:
        cfg, per_core = assign_slots(cls_of)
    except ValueError:
        cfg = tuple(["bf"] * NS)
        order = [i for i in range(B)]
        per_core = [order[c::N_CORES] for c in range(N_CORES)]

    noise_slots = [i for i, c in enumerate(cfg) if c in ("bf", "no")]
    band_slots = [i for i, c in enumerate(cfg) if c in ("bf", "fo")]
    n_noise = len(noise_slots)

    nc = _get_program(cfg)

    # device Qb = sum((g*x)^2) over the subsample; var(x1) ~ Qb*cq.
    # 'no' slots: ct = nm*std(x1) -> scale nm^2*cq
    # 'bf' slots: ct = nm*std(x1)/g (g folded into W) -> scale nm^2*cq/g^2
    cq = np.float32(T / (FSUB * P * (T - 1.0)))
    nm2cq = (nm * nm * cq).astype(np.float32)
    nm2cq_bf = (nm2cq / (g * g)).astype(np.float32)

    # bf16 interleaved views: xi16[s][p, j] = x[s, j*128 + p]
    xi16 = x.reshape(B, F, P).astype(ml_dtypes.bfloat16)
    ni16 = noise.reshape(B, F, P).astype(ml_dtypes.bfloat16)

    qi = np.arange(P)[:, None]
    pi = np.arange(P)[None, :]
    eye = (qi == pi)

    in_maps = []
    perm = np.empty(B, dtype=np.int64)
    for c in range(N_CORES):
        sl = np.array(per_core[c], dtype=np.int64)
        perm[c * NS:(c + 1) * NS] = sl
        xs = np.ascontiguousarray(xi16[sl].transpose(0, 2, 1))
        m = {"x_sh": xs}
        if n_noise:
            nsl = sl[noise_slots]
            m["n_sh"] = np.ascontiguousarray(ni16[nsl].transpose(0, 2, 1))
            # packed subsample for the stats prologue: [P, n_noise*FSUB],
            # xsub[p, j*FSUB + i] = x[s_j, (NSUB*i)*P + p]
            m["xsub"] = np.ascontiguousarray(
                xi16[nsl][:, ::NSUB, :].transpose(2, 0, 1).reshape(
                    P, n_noise * FSUB))
        if band_slots:
            ws = []
            for k in band_slots:
                si = sl[k]
                h, s, mm = int(h_eff[si]), s_coef[si], m_coef[si]
                gs = g[si]  # gain folded into the matrices
                w0 = gs * (s * (np.abs(qi - pi) <= h) + mm * eye)
                wl = (gs * s) * ((qi - pi) >= P - h)
                wu = (gs * s) * ((pi - qi) >= P - h)
                ws += [wl, w0, wu]
            m["w_sh"] = np.ascontiguousarray(
                np.concatenate(ws, axis=1).astype(ml_dtypes.bfloat16))
        scal = np.zeros((P, NS + max(n_noise, 1)), dtype=np.float32)
        scal[:, :NS] = g[sl][None, :]
        for j, k in enumerate(noise_slots):
            si = sl[k]
            scal[:, NS + j] = (nm2cq_bf if cfg[k] == "bf" else nm2cq)[si]
        m["scal"] = np.ascontiguousarray(scal)
        in_maps.append(m)

    res = run_bass_kernel_spmd(nc, in_maps, list(range(N_CORES)),
                               trace=_trace)
    LAST_RUN["exec_time_ns"] = res.exec_time_ns
    LAST_RUN["profile_json"] = res.profile_json

    out = np.empty((B, 1, T), dtype=np.float32)
    for c in range(N_CORES):
        y = res.results[c]["y_sh"]  # [NS, P, F] bf16
        y = np.asarray(y).transpose(0, 2, 1).reshape(NS, T).astype(np.float32)
        out[perm[c * NS:(c + 1) * NS], 0, :] = y
    return out
